# revision 71
# baseline (speedup 1.0000x reference)
"""Trainium2 Bass kernel v3 for nn_MCA_12214886990440 (strip-conv dual-axis attention).

Sharding: data-parallel over batch B=8 across 8 NeuronCores (params replicated).

BN is folded into conv weights host-side; x arrives pre-padded from DRAM
(borders hold -b/a so the folded BN contributes 0 there). Conv runs col-tiled
(2 pixel chunks concurrently on the PE column halves), 2 taps per pass
(K=128), at the fp16 PE roofline. q|v projections run "LDW-swapped": per
w-column the sc pixel-slab is the stationary operand and the weight matrix
streams, so psum lands directly in zqv's [h, (w, c)] layout with no staging
buffer and no xbar transposes; bias is seeded by a K=16 matmul (scalar-evac
banks) or fused into the vector evacuation. k goes through one xbar transpose
per branch. Attention is reassociated through 128x128 Grams:
    G_w[w2,w]   = sum_{d,h} hq[d,h,w2] wv[d,h,w]       (scale folded into Wq)
    w_o[w,(d,h)] = sum_{w2} G_w[w2,w] wk[d,h,w2]
and symmetrically for the h-branch. B^T runs per h-half with the head's 8
channels innermost (16B-coalesced fetch + evac); evacuations alternate
scalar/vector. The S pivot is 4 h-quarter xbar transposes on a dedicated sync
queue; projection pairs consecutive pixel chunks on the PE column halves,
y = x * sigmoid(wout@w_o + hout@h_o) is buffered in SBUF (fp16) and flushed
in 2 batched DMAs.
"""
import sys
sys.path.insert(0, "/opt/trn_rl_repo")

import numpy as np

import concourse.bass as bass
import concourse.tile as tile
from concourse import bacc
from concourse import mybir

B, C, H, W, NH, D = 8, 64, 128, 128, 8, 8
KS = [7, 11, 21]
EPS = 1e-5
PAD = 10
NTAP = 21
HW = H * W
PADROWS = H + 2 * PAD  # 148
F32 = mybir.dt.float32
F16 = mybir.dt.float16
AF = mybir.ActivationFunctionType
ALU = mybir.AluOpType

N_CORES = 8
CH = 512          # pixel chunk
NPAIR = 16        # chunk pairs (ci, ci+16)


DEBUG = False


def _kernel_body(tc, a):
    nc = tc.nc

    # ---------------- pools (alloc order = reverse release order) -----------
    wp = tc.alloc_tile_pool(name="wts", bufs=1)
    zp = tc.alloc_tile_pool(name="z", bufs=1)
    scp = tc.alloc_tile_pool(name="sc", bufs=1)
    chp = tc.alloc_tile_pool(name="chan", bufs=1)
    pp = tc.alloc_tile_pool(name="pad", bufs=1)

    # weights
    # weights in one fp16 blob + one fp32 bias DMA on the gpsimd queue: the
    # sync queue carries only the xbar transposes, and fewer DMA instructions
    # keep the semaphore pool from recycling mid-endgame
    wf16 = wp.tile([128, 3008], F16, tag="wf16", name="wf16")
    nc.gpsimd.dma_start(wf16[:], a["wf16"])
    wbias = wp.tile([128, 9], F32, tag="wbias", name="wbias")
    nc.gpsimd.dma_start(wbias[:], a["wbias"])
    wconv = wf16[:, 0:1408]
    wqkv = wf16[:, 1408:1856]
    bq2 = wf16[:, 1856:3008]
    convw = [wconv[:, 0:704], wconv[:, 704:1408]]
    qkv1w = [wqkv[:, 0:128], wqkv[:, 128:256]]
    qkv2w = [wqkv[:, 256:320], wqkv[:, 320:384]]
    projw = wqkv[:, 384:448]
    convb = [wbias[:, 0:1], wbias[:, 1:2]]
    qkv2b = [wbias[:, 4:5], wbias[:, 5:6]]
    projb = wbias[:, 6:7]

    # persistent pivoted tensors (fp16)
    zqv = [zp.tile([128, 128, 128], F16, tag=f"zqv{br}", name=f"zqv{br}")
           for br in range(2)]                       # [h, (w, c)]: c 0-63 q, 64-127 v
    zk = [zp.tile([128, 64, 128], F16, tag=f"zk{br}", name=f"zk{br}")
          for br in range(2)]                        # [w, (hrel, half*64+c)]

    # pads: host pre-padded (borders hold -b/a so folded BN gives 0 there);
    # both issued up front. Branch-1's chunks WAR-chase branch-0's conv reads
    # (range-based deps), so its fill overlaps branch-0 compute.
    pads = [pp.tile([128, PADROWS * W], F16, tag="pad", name=f"pad{br}")
            for br in range(2)]
    # pad-0 in 4 chunks, A-half rows first (conv runs A-chains first)
    NPC = PADROWS * W // 4
    for j in range(4):
        nc.scalar.dma_start(pads[0][:, j * NPC:(j + 1) * NPC],
                            a["padh"][:, j * NPC:(j + 1) * NPC])
    for j in range(4):
        nc.gpsimd.dma_start(pads[1][:, j * NPC:(j + 1) * NPC],
                            a["padw"][:, j * NPC:(j + 1) * NPC])

    # ---------------- phase A: conv + qkv per branch ----------------
    ps_conv = tc.alloc_tile_pool(name="ps_conv", bufs=2, space="PSUM")
    ps_qkv1 = tc.alloc_tile_pool(name="ps_qkv1", bufs=3, space="PSUM")
    ps_qkv2 = tc.alloc_tile_pool(name="ps_qkv2", bufs=1, space="PSUM")

    # warm the PE clock-gate while the pad DMAs land (output unused)
    wps0 = ps_conv.tile([128, CH], F32, tag="conv_a")
    for i in range(12):
        nc.tensor.matmul(wps0[:], wconv[:, 0:128], wconv[:, 0:CH],
                         start=(i == 0), stop=(i == 11))

    for br in range(2):  # 0 = h-branch (conv along H), 1 = w-branch
        sc = scp.tile([128, 16 * CH], F16, tag="sc", name=f"sc{br}")
        ck = chp.tile([128, 8192], F16, tag="ck", name=f"ck{br}")

        pad = pads[br]
        prr = pad[:].rearrange("p (h j) -> p h j", j=PADROWS)

        # conv: chunk pairs (ci, ci+16) col-tiled on PE column halves.
        # psum parts 0-63 = chunk ci out-chans, parts 64-127 = chunk ci+16.
        # A-chains and B-chains are emitted as independent accumulation
        # chains, A-first serpentine, so branch-0's A work starts as soon as
        # the first pad rows land instead of stalling on the B-half rows.
        cw = convw[br]

        def conv_rhs(base, g):
            if br == 0:
                return pad[:, (base + 2 * g) * W:(base + 2 * g) * W + CH]
            return prr[:, base:base + 4, 2 * g:2 * g + W]

        def conv_tail_rhs(base):
            if br == 0:
                return pad[0:64, (base + 20) * W:(base + 20) * W + CH]
            return prr[0:64, base:base + 4, 20:20 + W]

        # branch 0 staggers the B-half by 4 chains so conv starts on the A
        # rows while the B rows are still streaming in; branch 1's pad is
        # already resident, so no stagger (offset slots run at half rate)
        OFF = 2 if br == 0 else 0
        for k in range(NPAIR + OFF):
            a_ci = k if k < NPAIR else None
            b_ci = k - OFF if k >= OFF else None
            psA = (ps_conv.tile([128, CH], F32, tag="conv_a", name="psA")
                   if a_ci is not None else None)
            psB = (ps_conv.tile([128, CH], F32, tag="conv_b", name="psB")
                   if b_ci is not None else None)
            for g in range(10):
                w_g = cw[:, g * 64:(g + 1) * 64]
                if psA is not None:
                    nc.tensor.matmul(psA[0:64, :], w_g, conv_rhs(4 * a_ci, g),
                                     start=(g == 0), stop=False)
                if psB is not None:
                    nc.tensor.matmul(psB[64:128, :], w_g,
                                     conv_rhs(4 * b_ci + 64, g),
                                     start=(g == 0), stop=False)
            w_g = cw[0:64, 640:704]
            if psA is not None:
                nc.tensor.matmul(psA[0:64, :], w_g, conv_tail_rhs(4 * a_ci),
                                 start=False, stop=True)
                nc.scalar.activation(sc[0:64, a_ci * CH:(a_ci + 1) * CH],
                                     psA[0:64, :], AF.Identity,
                                     bias=convb[br][0:64, :])
            if psB is not None:
                nc.tensor.matmul(psB[64:128, :], w_g,
                                 conv_tail_rhs(4 * b_ci + 64),
                                 start=False, stop=True)
                nc.scalar.activation(sc[64:128, b_ci * CH:(b_ci + 1) * CH],
                                     psB[64:128, :], AF.Identity,
                                     bias=convb[br][64:128, :])

        # qkv2 (k, M=64): h-major pixel chunks (w inner 128) so the xbar
        # transpose lands w on partitions. ck parts 0-63: [c, (h 0-63, w)],
        # parts 64-127: [c, (h 64-127, w)].
        for ci in range(NPAIR):
            ps2 = ps_qkv2.tile([128, CH], F32, tag="qkv2", name="ps2")
            nc.tensor.matmul(ps2[0:64, :], qkv2w[br][0:64, :],
                             sc[0:64, ci * CH:(ci + 1) * CH],
                             start=True, stop=True)
            nc.tensor.matmul(ps2[64:128, :], qkv2w[br][64:128, :],
                             sc[64:128, ci * CH:(ci + 1) * CH],
                             start=True, stop=True)
            nc.vector.tensor_scalar_add(ck[0:64, ci * CH:(ci + 1) * CH],
                                        ps2[0:64, :], qkv2b[br][0:64, :])
            nc.scalar.activation(ck[64:128, ci * CH:(ci + 1) * CH],
                                 ps2[64:128, :], AF.Identity,
                                 bias=qkv2b[br][64:128, :])
        nc.sync.dma_start_transpose(zk[br][:], ck[:])

        # qkv1-direct (q|v): per w-column, the LHS is the sc pixel-column
        # [c_in, h] (LDWEIGHTS, strided load overlaps the previous stream)
        # and the RHS is the weight matrix [c_in, 128 q|v chans] (contiguous
        # fetch). Output psum [h, (w4, c128)] lands zqv's layout directly --
        # no cqv staging, no xbar transposes. Bias comes in as a K=1 matmul
        # against a host-packed bias row (it varies along the free dim, so
        # the activation-bias path can't apply it).
        scrA = sc[0:64, :].rearrange("c (q h w) -> c q h w", h=4, w=W)
        scrB = sc[64:128, :].rearrange("c (q h w) -> c q h w", h=4, w=W)
        ones16 = bq2[0:16, 1024:1152]   # 1/16-valued, K=16 for parallel fetch
        brow = bq2[0:16, br * 512:(br + 1) * 512]
        bfull = bq2[:, br * 512:(br + 1) * 512].rearrange(
            "p (w c) -> p w c", w=4)
        for wb in range(32):  # 4 w-columns per psum bank
            qps = ps_qkv1.tile([128, CH], F32, tag="qv")
            scalar_evac = (wb % 2 == 0)
            if scalar_evac:
                # bias seeded by a K=16 matmul; evacuated by scalar ACT copy
                nc.tensor.matmul(qps[:], ones16, brow, start=True, stop=False)
            for i in range(4):
                w = 4 * wb + i
                nc.tensor.matmul(qps[0:64, i * 128:(i + 1) * 128],
                                 scrA[:, :, :, w], qkv1w[br][0:64, :],
                                 start=(not scalar_evac and i == 0),
                                 stop=(i == 3))
                nc.tensor.matmul(qps[64:128, i * 128:(i + 1) * 128],
                                 scrB[:, :, :, w], qkv1w[br][64:128, :],
                                 start=(not scalar_evac and i == 0),
                                 stop=(i == 3))
            dst = zqv[br][:, 4 * wb:4 * wb + 4, :]
            src = qps[:].rearrange("p (w c) -> p w c", w=4)
            if scalar_evac:
                nc.scalar.activation(dst, src, AF.Copy)
            else:
                # bias fused into the vector evacuation
                nc.vector.tensor_add(dst, src, bfull)

        if DEBUG:
            nc.sync.dma_start(a[f"dbg_sc{br}"], sc[:])
            nc.sync.dma_start(a[f"dbg_zqv{br}"],
                              zqv[br][:].rearrange("h w c -> h (w c)"))
            nc.sync.dma_start(a[f"dbg_zk{br}"],
                              zk[br][:].rearrange("w r b -> w (r b)"))

    ps_qkv2.release()
    ps_qkv1.release()
    ps_conv.release()
    pp.release()
    chp.release()
    scp.release()

    # ---------------- phase B: attention ----------------
    gp = tc.alloc_tile_pool(name="g", bufs=1)
    zsp = tc.alloc_tile_pool(name="zs", bufs=1)
    sp = tc.alloc_tile_pool(name="s", bufs=1)
    rp = tc.alloc_tile_pool(name="ring", bufs=2)
    xpfp = tc.alloc_tile_pool(name="xpfp", bufs=1)
    ps_g = tc.alloc_tile_pool(name="ps_g", bufs=3, space="PSUM")
    ps_bt = tc.alloc_tile_pool(name="ps_bt", bufs=3, space="PSUM")
    ps_pj = tc.alloc_tile_pool(name="ps_pj", bufs=2, space="PSUM")

    gsb = gp.tile([128, 16 * 128], F16, tag="gsb", name="gsb")
    zs = zsp.tile([128, 16384], F16, tag="zs", name="zs")   # [w, (h, c)]
    s_cp = sp.tile([128, 128, 128], F16, tag="scp", name="scp")  # [c, h, w]

    # prefetch x (fp16) for the final multiply; on the gpsimd queue so the
    # sync queue stays clear for the S-pivot transposes. Even pixel chunks
    # land on parts 0:64, odd on 64:128, matching the projection pairing.
    xpf = xpfp.tile([128, 8192], F16, tag="xpf")
    ytf = xpfp.tile([128, 8192], F16, tag="ytf", name="ytf")
    x16r = a["x16"].rearrange("c (i u) -> c i u", u=512)
    nc.gpsimd.dma_start(xpf[0:64, :].rearrange("c (i u) -> c i u", u=512),
                        x16r[:, 0:32:2, :])
    nc.gpsimd.dma_start(xpf[64:128, :].rearrange("c (i u) -> c i u", u=512),
                        x16r[:, 1:32:2, :])

    # Gram + B^T fused per head (B^T follows its head's Gram immediately
    # so the S pivot can start right after the last head instead of a full
    # B^T phase later).
    # B^T rhs streams (s, r, c)-order: the 4 head-channels are innermost so
    # the PE fetches 8B-coalesced chunks instead of lone fp16 elements.
    # Evacuations alternate scalar/vector: the strided dst (4-elem chunks
    # every 256B) is slow on any one engine (~2.7us), so a single engine
    # serializes the whole phase.
    evac_idx = [0]

    def bt_evac(dst, src):
        k = evac_idx[0] % 2
        evac_idx[0] += 1
        if k == 0:
            nc.scalar.activation(dst, src, AF.Copy)
        else:
            nc.vector.tensor_copy(dst, src)

    for gi in range(2):
        zq = zqv[0] if gi == 0 else zqv[1]
        zv = zqv[1] if gi == 0 else zqv[0]
        zkk = zk[1] if gi == 0 else zk[0]   # w_o uses wk; h_o uses hk

        for n in range(NH):
            gps = ps_g.tile([128, CH], F32, tag="g")
            for d in range(D):
                c = n * D + d
                lhs = zq[:, :, c:c + 1].rearrange("h w e -> h (w e)")
                rhs = zv[:, :, 64 + c:65 + c].rearrange("h w e -> h (w e)")
                nc.tensor.matmul(gps[:, 0:128], lhs, rhs,
                                 start=(d == 0), stop=(d == D - 1))
            g_ap = gsb[:, (gi * NH + n) * 128:(gi * NH + n + 1) * 128]
            nc.scalar.activation(g_ap, gps[:, 0:128], AF.Copy)
            # B^T per h-half: rhs [w2, (r64, c8)] streams the head's full 8
            # channels innermost -> 16B-coalesced fetches, and the evac dst
            # writes 16B chunks instead of 8B
            for s in range(2):
                bps = ps_bt.tile([128, CH], F32, tag="bt")
                rhs = zkk[:, :, s * 64 + n * D:s * 64 + n * D + 8]
                nc.tensor.matmul(bps[:], g_ap, rhs, start=True, stop=True)
                zh = zs[:, s * 8192:(s + 1) * 8192].rearrange(
                    "w (r c) -> w r c", c=128)
                bt_evac(zh[:, :, gi * 64 + n * D:gi * 64 + n * D + 8],
                        bps[:].rearrange("w (r c) -> w r c", r=64))

    if DEBUG:
        nc.sync.dma_start(a["dbg_gsb"], gsb[:])
        nc.sync.dma_start(a["dbg_zs"], zs[:, 0:8192])

    # S pivot: [w, (h, c)] -> [c, h, w]; 4 h-quarter transposes (1MB each
    # runs the xbar near peak, and 4+4 endgame DMAs stay inside the DMA
    # semaphore pool -- 8 eighths pushed it to 12 and the last pivot
    # inherited a recycled-semaphore wait on a y flush)
    for q in range(4):
        nc.sync.dma_start_transpose(
            s_cp[:, q * 32:(q + 1) * 32, :], zs[:, q * 4096:(q + 1) * 4096])

    if DEBUG:
        nc.sync.dma_start(a["dbg_scp"], s_cp[:].rearrange("c h w -> c (h w)"))

    # projection + sigmoid + x*sig -> y. Col-group pairs are CONSECUTIVE
    # pixel chunks (2ci, 2ci+1) so chunk ci only needs S-pivot quarter ci//4
    # -- the old (ci, ci+16) pairing made every chunk wait for late pivots.
    s_flat = s_cp[:].rearrange("c a b -> c (a b)")
    yre = a["y"].rearrange("c (i u) -> c i u", u=512)
    for ci in range(NPAIR):
        pps = ps_pj.tile([128, CH], F32, tag="pj")
        nc.tensor.matmul(pps[0:64, :], projw,
                         s_flat[:, (2 * ci) * CH:(2 * ci + 1) * CH],
                         start=True, stop=True)
        nc.tensor.matmul(pps[64:128, :], projw,
                         s_flat[:, (2 * ci + 1) * CH:(2 * ci + 2) * CH],
                         start=True, stop=True)
        sg = rp.tile([128, CH], F32, tag="sg")
        nc.scalar.activation(sg[:], pps[:], AF.Sigmoid, bias=projb)
        nc.vector.tensor_mul(ytf[:, ci * CH:(ci + 1) * CH], sg[:],
                             xpf[:, ci * CH:(ci + 1) * CH])
        if ci % 8 == 7:
            # y flushed in 2 half-image batches on gpsimd; small frequent
            # y-DMAs would serialize the S-pivots through the shared DMA
            # semaphore pool
            c0 = ci - 7
            ytr = ytf[:].rearrange("c (i u) -> c i u", u=512)
            nc.gpsimd.dma_start(yre[:, 2 * c0:2 * ci + 2:2, :],
                                ytr[0:64, c0:ci + 1, :])
            nc.gpsimd.dma_start(yre[:, 2 * c0 + 1:2 * ci + 2:2, :],
                                ytr[64:128, c0:ci + 1, :])

    for p in (ps_pj, ps_bt, ps_g, xpfp, rp, sp, zsp, gp, zp, wp):
        p.release()


def _prep_weights(inputs):
    """Host-side packing: BN folded into conv weights, qkv biases folded."""
    inp = {k: np.asarray(v, dtype=np.float64) for k, v in inputs.items()}
    w = {}
    a1 = inp["bn1_g"] / np.sqrt(inp["bn1_v"] + EPS)
    b1 = inp["bn1_b"] - inp["bn1_m"] * a1
    a2 = inp["bn2_g"] / np.sqrt(inp["bn2_v"] + EPS)
    b2 = inp["bn2_b"] - inp["bn2_m"] * a2

    def conv_pack(ws, ab, bb, bias):
        # eff[t][o, i]; BN: x_bn = a*x + b folded: W' = W*diag(a), b' += sum_t W_t@b
        eff = np.zeros((NTAP, C, C))
        for j, k in enumerate(KS):
            off = PAD - k // 2
            for i in range(k):
                eff[off + i] += ws[j][:, :, i]
        bconv = bias + sum(eff[t] @ bb for t in range(NTAP))
        effs = eff * ab[None, None, :]
        pk = np.zeros((128, 704))
        for g in range(10):
            pk[0:64, g * 64:(g + 1) * 64] = effs[2 * g].T
            pk[64:128, g * 64:(g + 1) * 64] = effs[2 * g + 1].T
        pk[0:64, 640:704] = effs[20].T
        return pk, bconv

    pk_h, bc_h = conv_pack([inp[f"sc1_w{j}"][:, :, :, 0] for j in range(3)],
                           a1, b1, inp["sc1_b0"] + inp["sc1_b1"] + inp["sc1_b2"])
    pk_w, bc_w = conv_pack([inp[f"sc2_w{j}"][:, :, 0, :] for j in range(3)],
                           a2, b2, inp["sc2_b0"] + inp["sc2_b1"] + inp["sc2_b2"])

    scale = D * H ** (-0.5)
    idx = (np.arange(NH)[:, None] * 24 + np.arange(D)[None, :]).ravel()
    idx_q, idx_k, idx_v = idx, idx + 8, idx + 16

    wqkv = np.zeros((128, 448))
    wbias = np.zeros((128, 9))
    wbias[:, 0] = np.tile(bc_h, 2)
    wbias[:, 1] = np.tile(bc_w, 2)
    for br, (qw, qb, bc) in enumerate(
            [(inp["hqkv_w"], inp["hqkv_b"], bc_h),
             (inp["wqkv_w"], inp["wqkv_b"], bc_w)]):
        bfold = qb
        Wq, Wk, Wv = qw[idx_q] * scale, qw[idx_k], qw[idx_v]
        bq, bk, bv = bfold[idx_q] * scale, bfold[idx_k], bfold[idx_v]
        q1 = np.concatenate([Wq.T, Wv.T], axis=1)          # [64, 128]
        wqkv[:, br * 128:(br + 1) * 128] = np.tile(q1, (2, 1))
        wqkv[:, 256 + br * 64:256 + (br + 1) * 64] = np.tile(Wk.T, (2, 1))
        wbias[:, 2 + br] = np.concatenate([bq, bv])
        wbias[:, 4 + br] = np.tile(bk, 2)
    wqkv[:, 384:448] = np.concatenate([inp["wout_w"].T, inp["hout_w"].T],
                                      axis=0)              # [128, 64]
    wbias[:, 6] = np.tile(inp["wout_b"] + inp["hout_b"], 2)
    wbias[:, 7] = np.tile(-b1 / a1, 2)
    wbias[:, 8] = np.tile(-b2 / a2, 2)

    wconv = np.concatenate([pk_h, pk_w], axis=1)           # [128, 1408]

    # qkv1-direct bias row: [wi4, c128] pattern per branch + ones for the
    # K=1 bias matmul lhs
    bq2 = np.zeros((128, 1152))
    bq2[:, 0:512] = np.tile(wbias[:, 2], 4)[None, :]
    bq2[:, 512:1024] = np.tile(wbias[:, 3], 4)[None, :]
    bq2[:, 1024:1152] = 1.0 / 16.0  # K=16 bias-matmul lhs

    wf16 = np.concatenate([wconv, wqkv, bq2], axis=1)      # [128, 3008]
    return {"wf16": wf16.astype(np.float16),
            "wbias": wbias.astype(np.float32),
            "nbh": (-b1 / a1), "nbw": (-b2 / a2)}


_NC_CACHE = {}
_RUN_OPTS = {"trace": False}
_LAST_RESULT = {}

_SHAPES = {"x16": ([C, HW], F16),
           "wf16": ([128, 3008], F16), "wbias": ([128, 9], F32),
           "padh": ([128, PADROWS * W], F16), "padw": ([128, PADROWS * W], F16)}


def _build_nc():
    if "nc" in _NC_CACHE:
        return _NC_CACHE["nc"]
    nc = bacc.Bacc(trn_type="TRN2", target_bir_lowering=False, debug=False)
    a = {}
    for n, (s, dt) in _SHAPES.items():
        a[n] = nc.dram_tensor(n, s, dt, kind="ExternalInput").ap()
    a["y"] = nc.dram_tensor("y", [C, HW], F16, kind="ExternalOutput").ap()
    if _kernel_body.__globals__["DEBUG"]:
        dbg = {"dbg_sc0": [128, 8192], "dbg_sc1": [128, 8192],
               "dbg_zqv0": [128, HW], "dbg_zqv1": [128, HW],
               "dbg_zk0": [128, 8192], "dbg_zk1": [128, 8192],
               "dbg_gsb": [128, 2048], "dbg_zs": [128, 8192],
               "dbg_scp": [128, HW]}
        for n, s in dbg.items():
            a[n] = nc.dram_tensor(n, s, F16, kind="ExternalOutput").ap()
    with tile.TileContext(nc) as tc:
        _kernel_body(tc, a)
    nc.compile()
    _NC_CACHE["nc"] = nc
    return nc


def _in_maps(inputs):
    w = _prep_weights(inputs)
    nbh, nbw = w.pop("nbh"), w.pop("nbw")
    x = np.ascontiguousarray(np.asarray(inputs["x"], dtype=np.float32))
    maps = []
    for core in range(N_CORES):
        xc = x[core].reshape(C, H, W).astype(np.float16)
        # padh: [c|c-shifted, (hpad 148, w)]; border rows hold -b/a
        ph = np.empty((128, PADROWS, W), np.float16)
        ph[0:64] = nbh[:, None, None]
        ph[64:128] = nbh[:, None, None]
        ph[0:64, PAD:PAD + H, :] = xc
        ph[64:128, PAD - 1:PAD - 1 + H, :] = xc
        # padw: [c|c-shifted, (h, wpad 148)]; border cols hold -b/a
        pw = np.empty((128, H, PADROWS), np.float16)
        pw[0:64] = nbw[:, None, None]
        pw[64:128] = nbw[:, None, None]
        pw[0:64, :, PAD:PAD + H] = xc
        pw[64:128, :, PAD - 1:PAD - 1 + H] = xc
        m = {"x16": np.ascontiguousarray(xc.reshape(C, HW)),
             "padh": np.ascontiguousarray(ph.reshape(128, PADROWS * W)),
             "padw": np.ascontiguousarray(pw.reshape(128, PADROWS * W))}
        m.update(w)
        maps.append(m)
    return maps


def kernel(**inputs):
    from concourse.bass_utils import run_bass_kernel_spmd

    nc = _build_nc()
    res = run_bass_kernel_spmd(nc, _in_maps(inputs), core_ids=list(range(N_CORES)),
                               trace=_RUN_OPTS["trace"])
    _LAST_RESULT["res"] = res
    out = np.stack([res.results[i]["y"].reshape(C, H, W) for i in range(N_CORES)])
    return out.astype(np.float32)


if __name__ == "__main__":
    nc = _build_nc()
    print("built ok")



# revision 72
# speedup vs baseline: 1.0035x; 1.0035x over previous
"""Trainium2 Bass kernel v3 for nn_MCA_12214886990440 (strip-conv dual-axis attention).

Sharding: data-parallel over batch B=8 across 8 NeuronCores (params replicated).

BN is folded into conv weights host-side; x arrives pre-padded from DRAM
(borders hold -b/a so the folded BN contributes 0 there). Conv runs col-tiled
(2 pixel chunks concurrently on the PE column halves), 2 taps per pass
(K=128), at the fp16 PE roofline. q|v projections run "LDW-swapped": per
w-column the sc pixel-slab is the stationary operand and the weight matrix
streams, so psum lands directly in zqv's [h, (w, c)] layout with no staging
buffer and no xbar transposes; bias is seeded by a K=16 matmul (scalar-evac
banks) or fused into the vector evacuation. k goes through one xbar transpose
per branch. Attention is reassociated through 128x128 Grams:
    G_w[w2,w]   = sum_{d,h} hq[d,h,w2] wv[d,h,w]       (scale folded into Wq)
    w_o[w,(d,h)] = sum_{w2} G_w[w2,w] wk[d,h,w2]
and symmetrically for the h-branch. B^T runs per h-half with the head's 8
channels innermost (16B-coalesced fetch + evac); evacuations alternate
scalar/vector. The S pivot is 4 h-quarter xbar transposes on a dedicated sync
queue; projection pairs consecutive pixel chunks on the PE column halves,
y = x * sigmoid(wout@w_o + hout@h_o) is buffered in SBUF (fp16) and flushed
in 2 batched DMAs.
"""
import sys
sys.path.insert(0, "/opt/trn_rl_repo")

import numpy as np

import concourse.bass as bass
import concourse.tile as tile
from concourse import bacc
from concourse import mybir

B, C, H, W, NH, D = 8, 64, 128, 128, 8, 8
KS = [7, 11, 21]
EPS = 1e-5
PAD = 10
NTAP = 21
HW = H * W
PADROWS = H + 2 * PAD  # 148
F32 = mybir.dt.float32
F16 = mybir.dt.float16
AF = mybir.ActivationFunctionType
ALU = mybir.AluOpType

N_CORES = 8
CH = 512          # pixel chunk
NPAIR = 16        # chunk pairs (ci, ci+16)


DEBUG = False


def _kernel_body(tc, a):
    nc = tc.nc

    # ---------------- pools (alloc order = reverse release order) -----------
    wp = tc.alloc_tile_pool(name="wts", bufs=1)
    zp = tc.alloc_tile_pool(name="z", bufs=1)
    scp = tc.alloc_tile_pool(name="sc", bufs=1)
    chp = tc.alloc_tile_pool(name="chan", bufs=1)
    pp = tc.alloc_tile_pool(name="pad", bufs=1)

    # weights
    # weights in one fp16 blob + one fp32 bias DMA on the gpsimd queue: the
    # sync queue carries only the xbar transposes, and fewer DMA instructions
    # keep the semaphore pool from recycling mid-endgame
    wf16 = wp.tile([128, 3008], F16, tag="wf16", name="wf16")
    nc.gpsimd.dma_start(wf16[:], a["wf16"])
    wbias = wp.tile([128, 9], F32, tag="wbias", name="wbias")
    nc.gpsimd.dma_start(wbias[:], a["wbias"])
    wconv = wf16[:, 0:1408]
    wqkv = wf16[:, 1408:1856]
    bq2 = wf16[:, 1856:3008]
    convw = [wconv[:, 0:704], wconv[:, 704:1408]]
    qkv1w = [wqkv[:, 0:128], wqkv[:, 128:256]]
    qkv2w = [wqkv[:, 256:320], wqkv[:, 320:384]]
    projw = wqkv[:, 384:448]
    convb = [wbias[:, 0:1], wbias[:, 1:2]]
    qkv2b = [wbias[:, 4:5], wbias[:, 5:6]]
    projb = wbias[:, 6:7]

    # persistent pivoted tensors (fp16)
    zqv = [zp.tile([128, 128, 128], F16, tag=f"zqv{br}", name=f"zqv{br}")
           for br in range(2)]                       # [h, (w, c)]: c 0-63 q, 64-127 v
    zk = [zp.tile([128, 64, 128], F16, tag=f"zk{br}", name=f"zk{br}")
          for br in range(2)]                        # [w, (hrel, half*64+c)]

    # pads: host pre-padded (borders hold -b/a so folded BN gives 0 there);
    # both issued up front. Branch-1's chunks WAR-chase branch-0's conv reads
    # (range-based deps), so its fill overlaps branch-0 compute.
    pads = [pp.tile([128, PADROWS * W], F16, tag="pad", name=f"pad{br}")
            for br in range(2)]
    # pad-0 in 4 chunks, A-half rows first (conv runs A-chains first)
    NPC = PADROWS * W // 4
    for j in range(4):
        nc.scalar.dma_start(pads[0][:, j * NPC:(j + 1) * NPC],
                            a["padh"][:, j * NPC:(j + 1) * NPC])
    for j in range(4):
        nc.gpsimd.dma_start(pads[1][:, j * NPC:(j + 1) * NPC],
                            a["padw"][:, j * NPC:(j + 1) * NPC])

    # ---------------- phase A: conv + qkv per branch ----------------
    ps_conv = tc.alloc_tile_pool(name="ps_conv", bufs=2, space="PSUM")
    ps_qkv1 = tc.alloc_tile_pool(name="ps_qkv1", bufs=3, space="PSUM")
    ps_qkv2 = tc.alloc_tile_pool(name="ps_qkv2", bufs=1, space="PSUM")

    # warm the PE clock-gate while the pad DMAs land (output unused)
    wps0 = ps_conv.tile([128, CH], F32, tag="conv_a")
    for i in range(12):
        nc.tensor.matmul(wps0[:], wconv[:, 0:128], wconv[:, 0:CH],
                         start=(i == 0), stop=(i == 11))

    for br in range(2):  # 0 = h-branch (conv along H), 1 = w-branch
        sc = scp.tile([128, 16 * CH], F16, tag="sc", name=f"sc{br}")
        ck = chp.tile([128, 8192], F16, tag="ck", name=f"ck{br}")

        pad = pads[br]
        prr = pad[:].rearrange("p (h j) -> p h j", j=PADROWS)

        # conv: chunk pairs (ci, ci+16) col-tiled on PE column halves.
        # psum parts 0-63 = chunk ci out-chans, parts 64-127 = chunk ci+16.
        # A-chains and B-chains are emitted as independent accumulation
        # chains, A-first serpentine, so branch-0's A work starts as soon as
        # the first pad rows land instead of stalling on the B-half rows.
        cw = convw[br]

        def conv_rhs(base, g):
            if br == 0:
                return pad[:, (base + 2 * g) * W:(base + 2 * g) * W + CH]
            return prr[:, base:base + 4, 2 * g:2 * g + W]

        def conv_tail_rhs(base):
            if br == 0:
                return pad[0:64, (base + 20) * W:(base + 20) * W + CH]
            return prr[0:64, base:base + 4, 20:20 + W]

        # branch 0 staggers the B-half by 4 chains so conv starts on the A
        # rows while the B rows are still streaming in; branch 1's pad is
        # already resident, so no stagger (offset slots run at half rate)
        OFF = 4 if br == 0 else 0
        for k in range(NPAIR + OFF):
            a_ci = k if k < NPAIR else None
            b_ci = k - OFF if k >= OFF else None
            psA = (ps_conv.tile([128, CH], F32, tag="conv_a", name="psA")
                   if a_ci is not None else None)
            psB = (ps_conv.tile([128, CH], F32, tag="conv_b", name="psB")
                   if b_ci is not None else None)
            for g in range(10):
                w_g = cw[:, g * 64:(g + 1) * 64]
                if psA is not None:
                    nc.tensor.matmul(psA[0:64, :], w_g, conv_rhs(4 * a_ci, g),
                                     start=(g == 0), stop=False)
                if psB is not None:
                    nc.tensor.matmul(psB[64:128, :], w_g,
                                     conv_rhs(4 * b_ci + 64, g),
                                     start=(g == 0), stop=False)
            w_g = cw[0:64, 640:704]
            if psA is not None:
                nc.tensor.matmul(psA[0:64, :], w_g, conv_tail_rhs(4 * a_ci),
                                 start=False, stop=True)
                nc.scalar.activation(sc[0:64, a_ci * CH:(a_ci + 1) * CH],
                                     psA[0:64, :], AF.Identity,
                                     bias=convb[br][0:64, :])
            if psB is not None:
                nc.tensor.matmul(psB[64:128, :], w_g,
                                 conv_tail_rhs(4 * b_ci + 64),
                                 start=False, stop=True)
                nc.scalar.activation(sc[64:128, b_ci * CH:(b_ci + 1) * CH],
                                     psB[64:128, :], AF.Identity,
                                     bias=convb[br][64:128, :])

        # qkv2 (k, M=64): h-major pixel chunks (w inner 128) so the xbar
        # transpose lands w on partitions. ck parts 0-63: [c, (h 0-63, w)],
        # parts 64-127: [c, (h 64-127, w)].
        for ci in range(NPAIR):
            ps2 = ps_qkv2.tile([128, CH], F32, tag="qkv2", name="ps2")
            nc.tensor.matmul(ps2[0:64, :], qkv2w[br][0:64, :],
                             sc[0:64, ci * CH:(ci + 1) * CH],
                             start=True, stop=True)
            nc.tensor.matmul(ps2[64:128, :], qkv2w[br][64:128, :],
                             sc[64:128, ci * CH:(ci + 1) * CH],
                             start=True, stop=True)
            nc.vector.tensor_scalar_add(ck[0:64, ci * CH:(ci + 1) * CH],
                                        ps2[0:64, :], qkv2b[br][0:64, :])
            nc.scalar.activation(ck[64:128, ci * CH:(ci + 1) * CH],
                                 ps2[64:128, :], AF.Identity,
                                 bias=qkv2b[br][64:128, :])
        nc.sync.dma_start_transpose(zk[br][:], ck[:])

        # qkv1-direct (q|v): per w-column, the LHS is the sc pixel-column
        # [c_in, h] (LDWEIGHTS, strided load overlaps the previous stream)
        # and the RHS is the weight matrix [c_in, 128 q|v chans] (contiguous
        # fetch). Output psum [h, (w4, c128)] lands zqv's layout directly --
        # no cqv staging, no xbar transposes. Bias comes in as a K=1 matmul
        # against a host-packed bias row (it varies along the free dim, so
        # the activation-bias path can't apply it).
        scrA = sc[0:64, :].rearrange("c (q h w) -> c q h w", h=4, w=W)
        scrB = sc[64:128, :].rearrange("c (q h w) -> c q h w", h=4, w=W)
        ones16 = bq2[0:16, 1024:1152]   # 1/16-valued, K=16 for parallel fetch
        brow = bq2[0:16, br * 512:(br + 1) * 512]
        bfull = bq2[:, br * 512:(br + 1) * 512].rearrange(
            "p (w c) -> p w c", w=4)
        for wb in range(32):  # 4 w-columns per psum bank
            qps = ps_qkv1.tile([128, CH], F32, tag="qv")
            scalar_evac = (wb % 2 == 0)
            if scalar_evac:
                # bias seeded by a K=16 matmul; evacuated by scalar ACT copy
                nc.tensor.matmul(qps[:], ones16, brow, start=True, stop=False)
            for i in range(4):
                w = 4 * wb + i
                nc.tensor.matmul(qps[0:64, i * 128:(i + 1) * 128],
                                 scrA[:, :, :, w], qkv1w[br][0:64, :],
                                 start=(not scalar_evac and i == 0),
                                 stop=(i == 3))
                nc.tensor.matmul(qps[64:128, i * 128:(i + 1) * 128],
                                 scrB[:, :, :, w], qkv1w[br][64:128, :],
                                 start=(not scalar_evac and i == 0),
                                 stop=(i == 3))
            dst = zqv[br][:, 4 * wb:4 * wb + 4, :]
            src = qps[:].rearrange("p (w c) -> p w c", w=4)
            if scalar_evac:
                nc.scalar.activation(dst, src, AF.Copy)
            else:
                # bias fused into the vector evacuation
                nc.vector.tensor_add(dst, src, bfull)

        if DEBUG:
            nc.sync.dma_start(a[f"dbg_sc{br}"], sc[:])
            nc.sync.dma_start(a[f"dbg_zqv{br}"],
                              zqv[br][:].rearrange("h w c -> h (w c)"))
            nc.sync.dma_start(a[f"dbg_zk{br}"],
                              zk[br][:].rearrange("w r b -> w (r b)"))

    ps_qkv2.release()
    ps_qkv1.release()
    ps_conv.release()
    pp.release()
    chp.release()
    scp.release()

    # ---------------- phase B: attention ----------------
    gp = tc.alloc_tile_pool(name="g", bufs=1)
    zsp = tc.alloc_tile_pool(name="zs", bufs=1)
    sp = tc.alloc_tile_pool(name="s", bufs=1)
    rp = tc.alloc_tile_pool(name="ring", bufs=2)
    xpfp = tc.alloc_tile_pool(name="xpfp", bufs=1)
    ps_g = tc.alloc_tile_pool(name="ps_g", bufs=3, space="PSUM")
    ps_bt = tc.alloc_tile_pool(name="ps_bt", bufs=3, space="PSUM")
    ps_pj = tc.alloc_tile_pool(name="ps_pj", bufs=2, space="PSUM")

    gsb = gp.tile([128, 16 * 128], F16, tag="gsb", name="gsb")
    zs = zsp.tile([128, 16384], F16, tag="zs", name="zs")   # [w, (h, c)]
    s_cp = sp.tile([128, 128, 128], F16, tag="scp", name="scp")  # [c, h, w]

    # prefetch x (fp16) for the final multiply; on the gpsimd queue so the
    # sync queue stays clear for the S-pivot transposes. Even pixel chunks
    # land on parts 0:64, odd on 64:128, matching the projection pairing.
    xpf = xpfp.tile([128, 8192], F16, tag="xpf")
    ytf = xpfp.tile([128, 8192], F16, tag="ytf", name="ytf")
    x16r = a["x16"].rearrange("c (i u) -> c i u", u=512)
    nc.gpsimd.dma_start(xpf[0:64, :].rearrange("c (i u) -> c i u", u=512),
                        x16r[:, 0:32:2, :])
    nc.gpsimd.dma_start(xpf[64:128, :].rearrange("c (i u) -> c i u", u=512),
                        x16r[:, 1:32:2, :])

    # Gram + B^T fused per head (B^T follows its head's Gram immediately
    # so the S pivot can start right after the last head instead of a full
    # B^T phase later).
    # B^T rhs streams (s, r, c)-order: the 4 head-channels are innermost so
    # the PE fetches 8B-coalesced chunks instead of lone fp16 elements.
    # Evacuations alternate scalar/vector: the strided dst (4-elem chunks
    # every 256B) is slow on any one engine (~2.7us), so a single engine
    # serializes the whole phase.
    evac_idx = [0]

    def bt_evac(dst, src):
        k = evac_idx[0] % 2
        evac_idx[0] += 1
        if k == 0:
            nc.scalar.activation(dst, src, AF.Copy)
        else:
            nc.vector.tensor_copy(dst, src)

    for gi in range(2):
        zq = zqv[0] if gi == 0 else zqv[1]
        zv = zqv[1] if gi == 0 else zqv[0]
        zkk = zk[1] if gi == 0 else zk[0]   # w_o uses wk; h_o uses hk

        for n in range(NH):
            gps = ps_g.tile([128, CH], F32, tag="g")
            for d in range(D):
                c = n * D + d
                lhs = zq[:, :, c:c + 1].rearrange("h w e -> h (w e)")
                rhs = zv[:, :, 64 + c:65 + c].rearrange("h w e -> h (w e)")
                nc.tensor.matmul(gps[:, 0:128], lhs, rhs,
                                 start=(d == 0), stop=(d == D - 1))
            g_ap = gsb[:, (gi * NH + n) * 128:(gi * NH + n + 1) * 128]
            nc.scalar.activation(g_ap, gps[:, 0:128], AF.Copy)
            # B^T per h-half: rhs [w2, (r64, c8)] streams the head's full 8
            # channels innermost -> 16B-coalesced fetches, and the evac dst
            # writes 16B chunks instead of 8B
            for s in range(2):
                bps = ps_bt.tile([128, CH], F32, tag="bt")
                rhs = zkk[:, :, s * 64 + n * D:s * 64 + n * D + 8]
                nc.tensor.matmul(bps[:], g_ap, rhs, start=True, stop=True)
                zh = zs[:, s * 8192:(s + 1) * 8192].rearrange(
                    "w (r c) -> w r c", c=128)
                bt_evac(zh[:, :, gi * 64 + n * D:gi * 64 + n * D + 8],
                        bps[:].rearrange("w (r c) -> w r c", r=64))

    if DEBUG:
        nc.sync.dma_start(a["dbg_gsb"], gsb[:])
        nc.sync.dma_start(a["dbg_zs"], zs[:, 0:8192])

    # S pivot: [w, (h, c)] -> [c, h, w]; 4 h-quarter transposes (1MB each
    # runs the xbar near peak, and 4+4 endgame DMAs stay inside the DMA
    # semaphore pool -- 8 eighths pushed it to 12 and the last pivot
    # inherited a recycled-semaphore wait on a y flush)
    for q in range(4):
        nc.sync.dma_start_transpose(
            s_cp[:, q * 32:(q + 1) * 32, :], zs[:, q * 4096:(q + 1) * 4096])

    if DEBUG:
        nc.sync.dma_start(a["dbg_scp"], s_cp[:].rearrange("c h w -> c (h w)"))

    # projection + sigmoid + x*sig -> y. Col-group pairs are CONSECUTIVE
    # pixel chunks (2ci, 2ci+1) so chunk ci only needs S-pivot quarter ci//4
    # -- the old (ci, ci+16) pairing made every chunk wait for late pivots.
    s_flat = s_cp[:].rearrange("c a b -> c (a b)")
    yre = a["y"].rearrange("c (i u) -> c i u", u=512)
    for ci in range(NPAIR):
        pps = ps_pj.tile([128, CH], F32, tag="pj")
        nc.tensor.matmul(pps[0:64, :], projw,
                         s_flat[:, (2 * ci) * CH:(2 * ci + 1) * CH],
                         start=True, stop=True)
        nc.tensor.matmul(pps[64:128, :], projw,
                         s_flat[:, (2 * ci + 1) * CH:(2 * ci + 2) * CH],
                         start=True, stop=True)
        sg = rp.tile([128, CH], F32, tag="sg")
        nc.scalar.activation(sg[:], pps[:], AF.Sigmoid, bias=projb)
        nc.vector.tensor_mul(ytf[:, ci * CH:(ci + 1) * CH], sg[:],
                             xpf[:, ci * CH:(ci + 1) * CH])
        if ci % 8 == 7:
            # y flushed in 2 half-image batches on gpsimd; small frequent
            # y-DMAs would serialize the S-pivots through the shared DMA
            # semaphore pool
            c0 = ci - 7
            ytr = ytf[:].rearrange("c (i u) -> c i u", u=512)
            nc.gpsimd.dma_start(yre[:, 2 * c0:2 * ci + 2:2, :],
                                ytr[0:64, c0:ci + 1, :])
            nc.gpsimd.dma_start(yre[:, 2 * c0 + 1:2 * ci + 2:2, :],
                                ytr[64:128, c0:ci + 1, :])

    for p in (ps_pj, ps_bt, ps_g, xpfp, rp, sp, zsp, gp, zp, wp):
        p.release()


def _prep_weights(inputs):
    """Host-side packing: BN folded into conv weights, qkv biases folded."""
    inp = {k: np.asarray(v, dtype=np.float64) for k, v in inputs.items()}
    w = {}
    a1 = inp["bn1_g"] / np.sqrt(inp["bn1_v"] + EPS)
    b1 = inp["bn1_b"] - inp["bn1_m"] * a1
    a2 = inp["bn2_g"] / np.sqrt(inp["bn2_v"] + EPS)
    b2 = inp["bn2_b"] - inp["bn2_m"] * a2

    def conv_pack(ws, ab, bb, bias):
        # eff[t][o, i]; BN: x_bn = a*x + b folded: W' = W*diag(a), b' += sum_t W_t@b
        eff = np.zeros((NTAP, C, C))
        for j, k in enumerate(KS):
            off = PAD - k // 2
            for i in range(k):
                eff[off + i] += ws[j][:, :, i]
        bconv = bias + sum(eff[t] @ bb for t in range(NTAP))
        effs = eff * ab[None, None, :]
        pk = np.zeros((128, 704))
        for g in range(10):
            pk[0:64, g * 64:(g + 1) * 64] = effs[2 * g].T
            pk[64:128, g * 64:(g + 1) * 64] = effs[2 * g + 1].T
        pk[0:64, 640:704] = effs[20].T
        return pk, bconv

    pk_h, bc_h = conv_pack([inp[f"sc1_w{j}"][:, :, :, 0] for j in range(3)],
                           a1, b1, inp["sc1_b0"] + inp["sc1_b1"] + inp["sc1_b2"])
    pk_w, bc_w = conv_pack([inp[f"sc2_w{j}"][:, :, 0, :] for j in range(3)],
                           a2, b2, inp["sc2_b0"] + inp["sc2_b1"] + inp["sc2_b2"])

    scale = D * H ** (-0.5)
    idx = (np.arange(NH)[:, None] * 24 + np.arange(D)[None, :]).ravel()
    idx_q, idx_k, idx_v = idx, idx + 8, idx + 16

    wqkv = np.zeros((128, 448))
    wbias = np.zeros((128, 9))
    wbias[:, 0] = np.tile(bc_h, 2)
    wbias[:, 1] = np.tile(bc_w, 2)
    for br, (qw, qb, bc) in enumerate(
            [(inp["hqkv_w"], inp["hqkv_b"], bc_h),
             (inp["wqkv_w"], inp["wqkv_b"], bc_w)]):
        bfold = qb
        Wq, Wk, Wv = qw[idx_q] * scale, qw[idx_k], qw[idx_v]
        bq, bk, bv = bfold[idx_q] * scale, bfold[idx_k], bfold[idx_v]
        q1 = np.concatenate([Wq.T, Wv.T], axis=1)          # [64, 128]
        wqkv[:, br * 128:(br + 1) * 128] = np.tile(q1, (2, 1))
        wqkv[:, 256 + br * 64:256 + (br + 1) * 64] = np.tile(Wk.T, (2, 1))
        wbias[:, 2 + br] = np.concatenate([bq, bv])
        wbias[:, 4 + br] = np.tile(bk, 2)
    wqkv[:, 384:448] = np.concatenate([inp["wout_w"].T, inp["hout_w"].T],
                                      axis=0)              # [128, 64]
    wbias[:, 6] = np.tile(inp["wout_b"] + inp["hout_b"], 2)
    wbias[:, 7] = np.tile(-b1 / a1, 2)
    wbias[:, 8] = np.tile(-b2 / a2, 2)

    wconv = np.concatenate([pk_h, pk_w], axis=1)           # [128, 1408]

    # qkv1-direct bias row: [wi4, c128] pattern per branch + ones for the
    # K=1 bias matmul lhs
    bq2 = np.zeros((128, 1152))
    bq2[:, 0:512] = np.tile(wbias[:, 2], 4)[None, :]
    bq2[:, 512:1024] = np.tile(wbias[:, 3], 4)[None, :]
    bq2[:, 1024:1152] = 1.0 / 16.0  # K=16 bias-matmul lhs

    wf16 = np.concatenate([wconv, wqkv, bq2], axis=1)      # [128, 3008]
    return {"wf16": wf16.astype(np.float16),
            "wbias": wbias.astype(np.float32),
            "nbh": (-b1 / a1), "nbw": (-b2 / a2)}


_NC_CACHE = {}
_RUN_OPTS = {"trace": False}
_LAST_RESULT = {}

_SHAPES = {"x16": ([C, HW], F16),
           "wf16": ([128, 3008], F16), "wbias": ([128, 9], F32),
           "padh": ([128, PADROWS * W], F16), "padw": ([128, PADROWS * W], F16)}


def _build_nc():
    if "nc" in _NC_CACHE:
        return _NC_CACHE["nc"]
    nc = bacc.Bacc(trn_type="TRN2", target_bir_lowering=False, debug=False)
    a = {}
    for n, (s, dt) in _SHAPES.items():
        a[n] = nc.dram_tensor(n, s, dt, kind="ExternalInput").ap()
    a["y"] = nc.dram_tensor("y", [C, HW], F16, kind="ExternalOutput").ap()
    if _kernel_body.__globals__["DEBUG"]:
        dbg = {"dbg_sc0": [128, 8192], "dbg_sc1": [128, 8192],
               "dbg_zqv0": [128, HW], "dbg_zqv1": [128, HW],
               "dbg_zk0": [128, 8192], "dbg_zk1": [128, 8192],
               "dbg_gsb": [128, 2048], "dbg_zs": [128, 8192],
               "dbg_scp": [128, HW]}
        for n, s in dbg.items():
            a[n] = nc.dram_tensor(n, s, F16, kind="ExternalOutput").ap()
    with tile.TileContext(nc) as tc:
        _kernel_body(tc, a)
    nc.compile()
    _NC_CACHE["nc"] = nc
    return nc


def _in_maps(inputs):
    w = _prep_weights(inputs)
    nbh, nbw = w.pop("nbh"), w.pop("nbw")
    x = np.ascontiguousarray(np.asarray(inputs["x"], dtype=np.float32))
    maps = []
    for core in range(N_CORES):
        xc = x[core].reshape(C, H, W).astype(np.float16)
        # padh: [c|c-shifted, (hpad 148, w)]; border rows hold -b/a
        ph = np.empty((128, PADROWS, W), np.float16)
        ph[0:64] = nbh[:, None, None]
        ph[64:128] = nbh[:, None, None]
        ph[0:64, PAD:PAD + H, :] = xc
        ph[64:128, PAD - 1:PAD - 1 + H, :] = xc
        # padw: [c|c-shifted, (h, wpad 148)]; border cols hold -b/a
        pw = np.empty((128, H, PADROWS), np.float16)
        pw[0:64] = nbw[:, None, None]
        pw[64:128] = nbw[:, None, None]
        pw[0:64, :, PAD:PAD + H] = xc
        pw[64:128, :, PAD - 1:PAD - 1 + H] = xc
        m = {"x16": np.ascontiguousarray(xc.reshape(C, HW)),
             "padh": np.ascontiguousarray(ph.reshape(128, PADROWS * W)),
             "padw": np.ascontiguousarray(pw.reshape(128, PADROWS * W))}
        m.update(w)
        maps.append(m)
    return maps


def kernel(**inputs):
    from concourse.bass_utils import run_bass_kernel_spmd

    nc = _build_nc()
    res = run_bass_kernel_spmd(nc, _in_maps(inputs), core_ids=list(range(N_CORES)),
                               trace=_RUN_OPTS["trace"])
    _LAST_RESULT["res"] = res
    out = np.stack([res.results[i]["y"].reshape(C, H, W) for i in range(N_CORES)])
    return out.astype(np.float32)


if __name__ == "__main__":
    nc = _build_nc()
    print("built ok")

